# revision 1
# baseline (speedup 1.0000x reference)
"""CondConv (MoE-routing) block on 8 Trainium2 NeuronCores.

Computation per sample (see reference model):
  x1 = relu(bn1(conv1x1(x, mix(r1(x), w1))))          256 -> 128 ch
  x2 = relu(bn2(dwconv3x3(x1, mix(r2(x1), w2))))      128 ch depthwise
  out = concat([x1, x2], ch)

Sharding: data-parallel over batch (32 samples -> 4 per core); each core
holds the full (tiny) expert weight banks.

Per-core program, software-pipelined as A(s)=loads..conv1..routing2 and
B(s)=depthwise..stores, emitted A0,A1,B0,A2,B1,A3,B2,B3 so the PE
(in-order engine) always has conv1 of the next sample to chew on while
sample s's routing-2 chain resolves:
  - conv1 as PE matmuls in float32r (TF32-class): K=256 in 2 partition
    tiles, N in 2-chunk PSUM groups (448 cols per bank at 512-aligned
    offsets). BN1 scale is folded into w1 host-side; one ACT op per
    group evacuates with fused bias+ReLU and emits pool2 partial sums
    via accum_out.
  - routing: global pools split across DVE (reduce) and ACT
    (Copy+accum_out); logit/broadcast matmuls on PE (weights
    pre-transposed and pre-scaled by 1/HW host-side); sigmoid on ACT;
    expert-mix on DVE reading the broadcast weights straight from PSUM.
  - depthwise 3x3 on PE in float32r: 9 accumulating diag-matmuls per
    chunk, psum += diag(k2[:,t]) @ x1pad_shifted_t, reading 2-D strided
    views of a 58-wide zero-padded copy of x1 built on GPSIMD (so no
    border corrections are needed); DVE evacuates with fused BN2+ReLU
    (bias-add + max(0)).
  - all big DMAs ride the SP HWDGE queue; weights go first as 2 packed
    transfers so sample 0 is never stuck behind the batch stream.

float32r end-to-end error vs the fp32 reference is ~3e-4 relative
(absmax gate headroom is ~2 orders of magnitude).
"""
import os
import numpy as np

B, CIN, H, W = 32, 256, 56, 56
COUT = 256
INIT_C = 128
EXP_C = 128
NE = 4
BN_EPS = 1e-5
NCORES = 8
SPB = B // NCORES  # samples per core
HW = H * W  # 3136
GUARD = 57
NCHUNK = 7
CHUNK = HW // NCHUNK  # 448
ROWS = CHUNK // W  # 8 image rows per chunk

_DW_OFFS = [dh * W + dw for dh in (-1, 0, 1) for dw in (-1, 0, 1)]
_FIX_TAPS = [(0, 0), (3, 0), (6, 0), (2, W - 1), (5, W - 1), (8, W - 1)]

_prog_cache = {}


def _legalize_sync(nc, budget=1):
    """Hoist excess semaphore waits onto same-engine EventSemaphore carriers.

    TRN2 instruction encodings hold only ~1 wait + 1 update; the Tile
    scheduler in this snapshot can attach several waits to one
    instruction, which fails walrus codegen ("Too many sync wait
    commands").  A carrier is a pure "stall until sem >= v" processed by
    the same engine sequencer, so all waits still complete before the
    original instruction dispatches.  Same-engine waits must NOT be
    dropped: engines pipeline consecutive instructions, so they are real
    synchronization.
    """
    import bass_rust

    f = nc.m.functions[0]
    ctr = 0
    for blk in f.blocks:
        insts = list(blk.instructions)
        out = []
        changed = False
        for inst in insts:
            si = inst.sync_info
            if si is not None and type(inst).__name__ != "InstEventSemaphore":
                if len(si.on_wait) > budget:
                    n_excess = len(si.on_wait) - budget
                    excess = si.on_wait[:n_excess]
                    keep = si.on_wait[n_excess:]
                    for w in excess:
                        ctr += 1
                        ev = bass_rust.InstEventSemaphore(
                            name=f"waitcarrier-{ctr}",
                            engine=inst.engine,
                            sync_info=bass_rust.SyncInfo(on_wait=[w], on_update=[]),
                        )
                        nc.register_instruction(ev)
                        out.append(ev)
                    si.on_wait = keep
                    inst.sync_info = si
                    changed = True
            out.append(inst)
        if changed:
            blk.instructions = out



def _build_program():
    import concourse.bass as bass
    import concourse.tile as tile
    from concourse import mybir

    f32 = mybir.dt.float32
    f32r = mybir.dt.float32r
    AF = mybir.ActivationFunctionType
    ALU = mybir.AluOpType
    AX = mybir.AxisListType.X

    nc = bass.Bass("TRN2", target_bir_lowering=False, debug=False)

    x_d = nc.dram_tensor("x", [SPB, CIN, HW], f32r, kind="ExternalInput").ap()
    w1t_d = nc.dram_tensor("w1t", [2, NE, 128, 128], f32, kind="ExternalInput").ap()
    wpack_d = nc.dram_tensor("wpack", [128, 314], f32, kind="ExternalInput").ap()
    out_d = nc.dram_tensor("out", [SPB, COUT, HW], f32r, kind="ExternalOutput").ap()

    # 1-chunk PSUM groups (one bank each, 6 slots in flight)
    GROUPS = [(n, n + 1) for n in range(NCHUNK)]
    COLS_A = 4 * CHUNK  # 1792 (after G0+G1)

    with tile.TileContext(nc) as tc:
        with (
            tc.tile_pool(name="weights", bufs=1) as wpool,
            tc.tile_pool(name="big", bufs=3) as bpool,
            tc.tile_pool(name="big2", bufs=2) as bpool2,
            tc.tile_pool(name="small", bufs=2) as spool,
            tc.tile_pool(name="ps2", bufs=6, space="PSUM") as ppool,
            tc.tile_pool(name="psums", bufs=2, space="PSUM") as pspool,
        ):
            # ---- persistent weights (2 DMAs: big w1t + packed rest) ----
            w1t_sb = wpool.tile([128, 2 * NE * 128], f32, tag="w1t")
            nc.sync.dma_start(
                w1t_sb[:].rearrange("p (g n) -> p g n", g=2 * NE),
                w1t_d[:].rearrange("j e p n -> p (j e) n"),
            )
            wpack_sb = wpool.tile([128, 314], f32, tag="wpack")
            nc.sync.dma_start(wpack_sb[:], wpack_d[:])
            # warm the ACT table sets (Copy+Sigmoid) before real data arrives
            warm = wpool.tile([1, 1], f32, tag="warm")
            nc.vector.memset(warm[:], 0.0)
            nc.scalar.activation(warm[:], warm[:], AF.Copy, accum_out=None)
            nc.scalar.activation(warm[:], warm[:], AF.Sigmoid)
            ident_sb = wpack_sb[:, 0:128]
            w2f_sb = wpack_sb[:, 128:164]
            r1wt_a = wpack_sb[:, 164:168]
            r1wt_b = wpack_sb[:, 168:172]
            r2wt_sb = wpack_sb[:, 172:176]
            bnb1_sb = wpack_sb[:, 176:177]
            bnb2_sb = wpack_sb[:, 177:178]
            ones1_sb = wpack_sb[0:1, 178:306]
            r1b_sb = wpack_sb[0:1, 306:310]
            r2b_sb = wpack_sb[0:1, 310:314]

            def stageA(s):
                    # ---- load x shard in pieces (SP HWDGE) with ----
                    # ---- incremental pooling: DVE reduces xa pieces, ----
                    # ---- ACT Copy+accum pools xb pieces into scratch. ----
                    # Sample 0 uses quarters to shorten the cold-start chain.
                    npc = 4 if s == 0 else 2
                    PW = HW // npc
                    xa = bpool.tile([128, HW], f32r, tag="xa")
                    xb = bpool.tile([128, HW], f32r, tag="xb")
                    for i in range(npc):
                        nc.sync.dma_start(
                            xa[:, i * PW : (i + 1) * PW], x_d[s, 0:128, i * PW : (i + 1) * PW]
                        )
                        nc.sync.dma_start(
                            xb[:, i * PW : (i + 1) * PW], x_d[s, 128:256, i * PW : (i + 1) * PW]
                        )
                    x1flat = bpool.tile([128, HW], f32r, tag="x1flat")
                    p1p = spool.tile([128, 8], f32, tag="p1p")
                    for i in range(npc):
                        nc.vector.reduce_sum(
                            p1p[:, i : i + 1], xa[:, i * PW : (i + 1) * PW], AX
                        )
                        nc.scalar.activation(
                            x1flat[:, i * PW : (i + 1) * PW],
                            xb[:, i * PW : (i + 1) * PW],
                            AF.Copy, accum_out=p1p[:, npc + i : npc + i + 1],
                        )
                    p1 = spool.tile([128, 2], f32, tag="p1")
                    nc.vector.reduce_sum(p1[:, 0:1], p1p[:, 0:npc], AX)
                    nc.vector.reduce_sum(p1[:, 1:2], p1p[:, npc : 2 * npc], AX)

                    # ---- routing 1 ----
                    ps_r = pspool.tile([128, NE], f32, tag="ps_small", name="ps_r")
                    nc.tensor.matmul(ps_r[0:1, :], p1[:, 0:1], r1wt_a, start=True, stop=False)
                    nc.tensor.matmul(ps_r[0:1, :], p1[:, 1:2], r1wt_b, start=False, stop=True)
                    r1s = spool.tile([1, NE], f32, tag="r1s")
                    nc.vector.tensor_tensor(r1s[:], ps_r[0:1, :], r1b_sb, op=ALU.add)
                    nc.scalar.activation(r1s[:], r1s[:], AF.Sigmoid)
                    ps_rb = pspool.tile([128, NE], f32, tag="ps_small", name="ps_rb")
                    nc.tensor.matmul(ps_rb[:], ones1_sb, r1s[:], start=True, stop=True)
                    rb = ps_rb

                    # ---- mix k1T (DVE) ----
                    k1t = spool.tile([128, 256], f32r, tag="k1t")
                    for j in range(2):
                        dst = k1t[:, j * 128 : (j + 1) * 128]
                        w_of = lambda e: w1t_sb[:, (j * NE + e) * 128 : (j * NE + e + 1) * 128]
                        nc.vector.tensor_scalar(dst, w_of(0), rb[:, 0:1], None, ALU.mult)
                        for e in range(1, NE):
                            nc.vector.scalar_tensor_tensor(
                                dst, w_of(e), rb[:, e : e + 1], dst, ALU.mult, ALU.add
                            )

                    # ---- conv1 in 2-chunk PSUM groups + BN1+ReLU evac ----
                    p2cols = spool.tile([128, len(GROUPS)], f32, tag="p2cols")
                    x1flat_r = x1flat[:].rearrange("p (h w) -> p h w", w=W)
                    xpad = bpool2.tile([128, 58 * 58], f32r, tag="xpad")
                    xpad_r = xpad[:].rearrange("p (r c) -> p r c", c=58)
                    nc.gpsimd.memset(xpad[:, 0:58].bitcast(f32), 0.0)
                    nc.gpsimd.memset(xpad[:, 57 * 58 :].bitcast(f32), 0.0)
                    nc.gpsimd.memset(xpad_r[:, 1:57, 0:1].bitcast(f32), 0.0)
                    nc.gpsimd.memset(xpad_r[:, 1:57, 57:58].bitcast(f32), 0.0)
                    for g, (n0, n1) in enumerate(GROUPS):
                        ng = n1 - n0
                        ps = ppool.tile([128, 512], f32, tag="ps2", name=f"c1_{s}_{g}")
                        for n in range(n0, n1):
                            off = (n - n0) * 512
                            for j, xt in ((0, xa), (1, xb)):
                                nc.tensor.matmul(
                                    ps[:, off : off + CHUNK],
                                    k1t[:, j * 128 : (j + 1) * 128],
                                    xt[:, n * CHUNK : (n + 1) * CHUNK],
                                    start=(j == 0), stop=(j == 1),
                                )
                        nc.scalar.activation(
                            x1flat[:, n0 * CHUNK : n1 * CHUNK].rearrange(
                                "p (c b) -> p c b", b=CHUNK
                            ),
                            ps[:, 0 : ng * 512].rearrange("p (c b) -> p c b", b=512)[
                                :, :, 0:CHUNK
                            ],
                            AF.Relu, bias=bnb1_sb, accum_out=p2cols[:, g : g + 1],
                        )
                    nc.sync.dma_start(out_d[s, 0:INIT_C, 0:COLS_A], x1flat[:, 0:COLS_A])
                    nc.sync.dma_start(out_d[s, 0:INIT_C, COLS_A:HW], x1flat[:, COLS_A:HW])
                    nc.gpsimd.tensor_copy(
                        xpad_r[:, 1:33, 1:57], x1flat_r[:, 0:32, :]
                    )
                    nc.gpsimd.tensor_copy(
                        xpad_r[:, 33:57, 1:57], x1flat_r[:, 32:56, :]
                    )

                    # ---- routing 2 ----
                    p2 = spool.tile([128, 1], f32, tag="p2")
                    nc.vector.reduce_sum(p2[:], p2cols[:], AX)
                    ps_r2 = pspool.tile([128, NE], f32, tag="ps_small", name="ps_r2")
                    nc.tensor.matmul(ps_r2[0:1, :], p2[:], r2wt_sb, start=True, stop=True)
                    r2s = spool.tile([1, NE], f32, tag="r2s")
                    nc.vector.tensor_tensor(r2s[:], ps_r2[0:1, :], r2b_sb, op=ALU.add)
                    nc.scalar.activation(r2s[:], r2s[:], AF.Sigmoid)
                    ps_rb2 = pspool.tile([128, NE], f32, tag="ps_small", name="ps_rb2")
                    nc.tensor.matmul(ps_rb2[:], ones1_sb, r2s[:], start=True, stop=True)
                    rb2 = ps_rb2

                    # ---- mix k2 and diag kernels (DVE) ----
                    k2 = spool.tile([128, 9], f32, tag="k2")
                    nc.vector.tensor_scalar(k2[:], w2f_sb[:, 0:9], rb2[:, 0:1], None, ALU.mult)
                    for e in range(1, NE):
                        nc.vector.scalar_tensor_tensor(
                            k2[:], w2f_sb[:, e * 9 : (e + 1) * 9], rb2[:, e : e + 1], k2[:],
                            ALU.mult, ALU.add,
                        )
                    diag = spool.tile([128, 9 * 128], f32r, tag="diag")
                    for t in range(9):
                        nc.vector.tensor_scalar(
                            diag[:, t * 128 : (t + 1) * 128], ident_sb,
                            k2[:, t : t + 1], None, ALU.mult,
                        )

                    return xpad_r, diag

            def stageB(s, xpad_r, diag):
                    # ---- depthwise on PE (f32r) + BN2+ReLU (DVE) ----
                    x2 = bpool2.tile([128, HW], f32r, tag="x2")
                    for g, (n0, n1) in enumerate(GROUPS):
                        ng = n1 - n0
                        ps = ppool.tile([128, 512], f32, tag="ps2", name=f"dw_{s}_{g}")
                        for n in range(n0, n1):
                            off = (n - n0) * 512
                            for t in range(9):
                                dh, dw = t // 3 - 1, t % 3 - 1
                                rhs = xpad_r[
                                    :, n * ROWS + dh + 1 : n * ROWS + dh + 9, dw + 1 : dw + 57
                                ]
                                nc.tensor.matmul(
                                    ps[:, off : off + CHUNK],
                                    diag[:, t * 128 : (t + 1) * 128], rhs,
                                    start=(t == 0), stop=(t == 8),
                                )
                        nc.vector.tensor_scalar(
                            x2[:, n0 * CHUNK : n1 * CHUNK].rearrange(
                                "p (c b) -> p c b", b=CHUNK
                            ),
                            ps[:, 0 : ng * 512].rearrange("p (c b) -> p c b", b=512)[
                                :, :, 0:CHUNK
                            ],
                            bnb2_sb, 0.0, ALU.add, ALU.max,
                        )
                    for g, (n0, n1) in enumerate(GROUPS):
                        nc.sync.dma_start(
                            out_d[s, INIT_C:COUT, n0 * CHUNK : n1 * CHUNK],
                            x2[:, n0 * CHUNK : n1 * CHUNK],
                        )


            order_handles = {}
            for s in range(SPB):
                order_handles[s] = stageA(s)
                if s >= 1:
                    stageB(s - 1, *order_handles[s - 1])
            stageB(SPB - 1, *order_handles[SPB - 1])

    return nc


def _host_prep(x, r1_w, r1_b, w1, g1, b1, m1, v1, r2_w, r2_b, w2, g2, b2, m2, v2):
    inv1 = g1 / np.sqrt(v1 + BN_EPS)
    inv2 = g2 / np.sqrt(v2 + BN_EPS)
    bnb1 = (b1 - m1 * inv1).reshape(INIT_C, 1).astype(np.float32)
    bnb2 = (b2 - m2 * inv2).reshape(EXP_C, 1).astype(np.float32)
    # w1: [E, O, C, 1, 1] -> fold inv1 over O -> w1t[j, e, c_local, o]
    w1s = w1[:, :, :, 0, 0] * inv1[None, :, None]  # [E, O, C]
    w1t = np.ascontiguousarray(
        w1s.transpose(2, 0, 1).reshape(2, 128, NE, 128).transpose(0, 2, 1, 3)
    ).astype(np.float32)  # [2, E, 128c, 128o]
    # w2: [E, C, 1, 3, 3] -> fold inv2 over C -> [E, C, 9]
    w2f = (w2[:, :, 0, :, :] * inv2[None, :, None, None]).reshape(NE, EXP_C, 9)
    w2f = np.ascontiguousarray(w2f).astype(np.float32)
    wpack = np.zeros((128, 314), dtype=np.float32)
    wpack[:, 0:128] = np.eye(128, dtype=np.float32)
    wpack[:, 128:164] = w2f.transpose(1, 0, 2).reshape(128, 36)
    r1wt = np.ascontiguousarray(r1_w.T / HW).astype(np.float32)
    wpack[:, 164:168] = r1wt[0:128]
    wpack[:, 168:172] = r1wt[128:256]
    wpack[:, 172:176] = (r2_w.T / HW).astype(np.float32)
    wpack[:, 176:177] = bnb1
    wpack[:, 177:178] = bnb2
    wpack[0:4, 178:306] = 1.0
    wpack[0, 306:310] = r1_b.astype(np.float32)
    wpack[0, 310:314] = r2_b.astype(np.float32)
    common = {
        "w1t": w1t,
        "wpack": wpack,
    }
    return common


def kernel(**inputs):
    x = np.asarray(inputs["x"], dtype=np.float32)
    common = _host_prep(**{k: np.asarray(v) for k, v in inputs.items()})

    if "nc" not in _prog_cache:
        _prog_cache["nc"] = _build_program()
    nc = _prog_cache["nc"]
    sim_mode = bool(os.environ.get("BASS_KERNEL_SIM"))
    if not sim_mode and not _prog_cache.get("fixed"):
        _legalize_sync(nc)
        _prog_cache["fixed"] = True

    xs = x.reshape(NCORES, SPB, CIN, HW)
    in_maps = [dict(common, x=np.ascontiguousarray(xs[c])) for c in range(NCORES)]

    if sim_mode:
        from concourse.bass_interp import CoreSim

        sim = CoreSim(nc)
        for name, arr in in_maps[0].items():
            sim.tensor(name)[:] = arr
        sim.simulate()
        out = np.zeros((NCORES, SPB, COUT, HW), dtype=np.float32)
        out[0] = sim.tensor("out")
        return out.reshape(B, COUT, H, W)

    from concourse.bass_utils import run_bass_kernel_spmd

    res = run_bass_kernel_spmd(nc, in_maps, list(range(NCORES)))
    _prog_cache["last_results"] = res
    out = np.stack([res.results[c]["out"] for c in range(NCORES)])
    return out.reshape(B, COUT, H, W)



# revision 40
# speedup vs baseline: 1.5989x; 1.5989x over previous
"""CondConv (MoE-routing) block on 8 Trainium2 NeuronCores.

Computation per sample (see reference model):
  x1 = relu(bn1(conv1x1(x, mix(r1(x), w1))))          256 -> 128 ch
  x2 = relu(bn2(dwconv3x3(x1, mix(r2(x1), w2))))      128 ch depthwise
  out = concat([x1, x2], ch)

Sharding: data-parallel over batch (32 samples -> 4 per core); each core
holds the full (tiny) expert weight banks.

Per-core design (driven by the TimelineSim cost model):
  - I/O rides reduced dtypes: x input bf16, x1 output bf16, x2 output
    fp8e3 (e3m4; x2 values are small vs the global output scale, so the
    3% element error lands ~3e-3 normalized).  DMA is the bottleneck
    device (~360B/ns shared), so bytes == time.
  - x1/x2 are stored and DMA'd in a 58-wide zero-padded row layout
    (56 rows x 58 cols, contiguous); the host strips the 2 pad columns.
    That makes the depthwise input padding free and keeps every DMA
    descriptor contiguous (>=512B).
  - conv1 as bf16 PE matmuls, K=256 in 2 partition tiles, 7 chunks of
    448 pixels; ACT evacuates with fused bias+ReLU into the padded
    bf16 buffer.
  - depthwise 3x3 on PE in fp8e4 (e4m3) DoubleRow perf mode: tap PAIRS
    are packed as the two k-tiles of one matmul (stationary
    [128,2,128] diag pair, moving = two tap-shifted overlapping views
    of the padded fp8 image), so 9 taps cost 5 matmuls at 0.5
    cycles/row = 4.5x less PE time than 9 full-rate diag matmuls.
    GPSIMD converts the padded bf16 x1 to fp8 (one copy per sample).
  - routing-1 logits = r1w @ pooled(x) are computed WITHOUT pooling
    first: r1wT-stationary PE matmuls accumulate [4,448] partial logits
    over the x chunks as each DMA half lands, then one DVE reduce.
    routing-2 reuses the conv1 evacs' accum_out column sums.  Sigmoids
    on ACT; expert mixes and the 9 fp8 diag builds on DVE (diag slot 9
    stays zero to pair the odd 9th tap).
  - engine-queue discipline is the speed limit (in-order queues +
    scheduler follows emission order): loads are alone on the SP queue,
    stores issue from ACT behind the evacs they wait on, and emission
    interleaves blocks as L(s+1) C(s) P1(s+1) R2(s) B(s-1) so sample
    s+1's pool1/conv1 sit on PE ahead of sample s's routing2/dw which
    wait on s's evacuations.

Cost-model steady state per sample (ns): PE ~8700 (pool1 2600 + conv1
2600 + dw 3400), ACT ~8000, DMA ~7900, DVE ~6000, GPSIMD ~4800.
End-to-end error vs the fp32 reference is ~1.3e-2 normalized (dominated
by e4m3 quantization of x1/k2 in the depthwise), inside the 2e-2 gate.
"""
import os
import numpy as np

B, CIN, H, W = 32, 256, 56, 56
COUT = 256
INIT_C = 128
EXP_C = 128
NE = 4
BN_EPS = 1e-5
NCORES = 8
SPB = B // NCORES  # samples per core
HW = H * W  # 3136
PADW = 58
FLAT = PADW * PADW  # 3364
PROW = PADW + 2 * PADW * 0  # noqa - clarity below
G8 = 64  # fp8 pad-image guard elems each side (dw taps read +-59)
PSPAN = 56 * PADW  # 3248: padded rows 1..57 flat span
NCHUNK = 7
CHUNK = HW // NCHUNK  # 448 real pixels (conv1 chunking)
PCH = PSPAN // NCHUNK  # 464 padded pixels (dw / pool2 chunking)

# dw tap flat offsets in the 58-pitch padded image, and the pairing into
# DoubleRow k-tiles.  HW constraint (found empirically): the moving AP's
# k-tile stride must be 2-byte aligned, so taps are paired within parity
# classes (all deltas even).  SLOT_TAPS maps diag slots 0..8 to tap
# indices; slot 9 stays zero (odd tap count).
_OFFS = [dh * PADW + dw for dh in (-1, 0, 1) for dw in (-1, 0, 1)]
_SLOT_TAPS = [0, 2, 3, 5, 6, 8, 1, 4, 7]
# pairs of diag slots with the rhs delta between their taps
_PAIRS = [  # (slot_t0, tap_t0, delta)
    (0, 0, 2),      # taps -59, -57
    (2, 3, 2),      # taps -1, 1
    (4, 6, 2),      # taps 57, 59
    (6, 1, PADW),   # taps -58, 0
    (8, 7, 2),      # tap 58 + zero slot
]

_prog_cache = {}


def _legalize_sync(nc, budget=1):
    """Hoist excess semaphore waits onto same-engine EventSemaphore carriers.

    TRN2 instruction encodings hold only ~1 wait + 1 update; the Tile
    scheduler in this snapshot can attach several waits to one
    instruction, which fails walrus codegen ("Too many sync wait
    commands").  A carrier is a pure "stall until sem >= v" processed by
    the same engine sequencer, so all waits still complete before the
    original instruction dispatches.  Same-engine waits must NOT be
    dropped: engines pipeline consecutive instructions, so they are real
    synchronization.
    """
    import bass_rust

    f = nc.m.functions[0]
    ctr = 0
    for blk in f.blocks:
        insts = list(blk.instructions)
        out = []
        changed = False
        for inst in insts:
            si = inst.sync_info
            if si is not None and type(inst).__name__ != "InstEventSemaphore":
                if len(si.on_wait) > budget:
                    n_excess = len(si.on_wait) - budget
                    excess = si.on_wait[:n_excess]
                    keep = si.on_wait[n_excess:]
                    for w in excess:
                        ctr += 1
                        ev = bass_rust.InstEventSemaphore(
                            name=f"waitcarrier-{ctr}",
                            engine=inst.engine,
                            sync_info=bass_rust.SyncInfo(on_wait=[w], on_update=[]),
                        )
                        nc.register_instruction(ev)
                        out.append(ev)
                    si.on_wait = keep
                    inst.sync_info = si
                    changed = True
            out.append(inst)
        if changed:
            blk.instructions = out


def _build_program():
    import concourse.bass as bass
    import concourse.tile as tile
    from concourse import mybir
    from bass_rust import AP as RAP

    f32 = mybir.dt.float32
    bf16 = mybir.dt.bfloat16
    fp8 = mybir.dt.float8e4
    fp8e3 = mybir.dt.float8e3
    AF = mybir.ActivationFunctionType
    ALU = mybir.AluOpType
    AX = mybir.AxisListType.X
    PM = mybir.MatmulPerfMode

    nc = bass.Bass("TRN2", target_bir_lowering=False, debug=False)

    x_d = nc.dram_tensor("x", [SPB, CIN, HW], bf16, kind="ExternalInput").ap()
    w1t_d = nc.dram_tensor("w1t", [2, NE, 128, 128], bf16, kind="ExternalInput").ap()
    r1wt_d = nc.dram_tensor("r1wt", [128, 3 * NE], bf16, kind="ExternalInput").ap()
    wpack_d = nc.dram_tensor("wpack", [128, 315], f32, kind="ExternalInput").ap()
    id8_d = nc.dram_tensor("id8", [128, 128], fp8, kind="ExternalInput").ap()
    out1_d = nc.dram_tensor("out1", [SPB, INIT_C, PSPAN], bf16, kind="ExternalOutput").ap()
    out2_d = nc.dram_tensor("out2", [SPB, EXP_C, PSPAN], fp8e3, kind="ExternalOutput").ap()

    with tile.TileContext(nc) as tc:
        with (
            tc.tile_pool(name="weights", bufs=1) as wpool,
            tc.tile_pool(name="xin", bufs=3) as xpool,
            tc.tile_pool(name="mid", bufs=2) as mpool,
            tc.tile_pool(name="small", bufs=2) as spool,
            tc.tile_pool(name="cps", bufs=3, space="PSUM") as cpool,
            tc.tile_pool(name="dps", bufs=3, space="PSUM") as dpool,
            tc.tile_pool(name="sps", bufs=2, space="PSUM") as ppool,
        ):
            # ---- persistent weights ----
            w1t_sb = wpool.tile([128, 2 * NE * 128], bf16, tag="w1t")
            nc.sync.dma_start(
                w1t_sb[:].rearrange("p (g n) -> p g n", g=2 * NE),
                w1t_d[:].rearrange("j e p n -> p (j e) n"),
            )
            r1wt_sb = wpool.tile([128, 3 * NE], bf16, tag="r1wt")
            nc.sync.dma_start(r1wt_sb[:], r1wt_d[:])
            r2wtb_sb = r1wt_sb[:, 2 * NE : 3 * NE]
            wpack_sb = wpool.tile([128, 315], f32, tag="wpack")
            nc.sync.dma_start(wpack_sb[:], wpack_d[:])
            id8_sb = wpool.tile([128, 128], fp8, tag="id8")
            nc.sync.dma_start(id8_sb[:], id8_d[:])
            w2f_sb = wpack_sb[:, 0:36]
            r2wt_sb = wpack_sb[:, 36:40]
            bnb1_sb = wpack_sb[:, 44:45]
            bnb2_sb = wpack_sb[:, 45:46]
            r1b_col = wpack_sb[0:4, 46:47]
            ones1_sb = wpack_sb[0:1, 47:175]
            r2b_sb = wpack_sb[0:1, 175:179]
            ones4_sb = wpack_sb[0:4, 183:311]
            id4_sb = wpack_sb[0:4, 311:315]

            # warm the ACT table sets before real data arrives
            warm = wpool.tile([1, 1], f32, tag="warm")
            nc.vector.memset(warm[:], 0.0)
            nc.scalar.activation(warm[:], warm[:], AF.Copy, accum_out=None)
            nc.scalar.activation(warm[:], warm[:], AF.Sigmoid)

            # ---- persistent ping-pong images (borders zeroed once) ----
            xpadb = [
                wpool.tile([128, FLAT], bf16, tag=f"xpadb{i}", name=f"xpadb{i}")
                for i in range(2)
            ]
            xpad8 = [
                wpool.tile([128, G8 + FLAT + G8], fp8, tag=f"xpad8{i}", name=f"xpad8{i}")
                for i in range(2)
            ]
            diag = [
                wpool.tile([128, 10, 128], fp8, tag=f"diag{i}", name=f"diag{i}")
                for i in range(2)
            ]
            for i in range(2):
                xr = xpadb[i][:].rearrange("p (r c) -> p r c", c=PADW)
                # cols 0/57 of rows 1..57 must be zero (conv evac writes 1..56 only)
                nc.vector.memset(xr[:, 1:57, 0:1], 0.0)
                nc.vector.memset(xr[:, 1:57, 57:58], 0.0)
                # fp8 image: guards + top/bottom pad rows (cols come from convert)
                nc.vector.memset(xpad8[i][:, 0 : G8 + PADW], 0.0)
                nc.vector.memset(xpad8[i][:, G8 + 57 * PADW :], 0.0)
                nc.vector.memset(diag[i][:, 9, :], 0.0)

            def stageL(s):
                """input loads (the ONLY thing on the SP queue so they never
                head-block behind stores)."""
                x8 = xpool.tile([128, 2, HW], bf16, tag="x8", name=f"x8_{s}")
                xsrc = x_d[s].rearrange("(j p) n -> p j n", j=2)
                npc = 4 if s == 0 else 1
                PW = HW // npc
                for j in range(2):
                    for i in range(npc):
                        nc.sync.dma_start(
                            x8[:, j, i * PW : (i + 1) * PW],
                            xsrc[:, j, i * PW : (i + 1) * PW],
                        )
                return x8

            def stageP1(s, x8):
                """pool1 (PE matmul-accumulate) -> routing1 -> k1 mix."""

                # ---- routing 1: r1w @ pooled computed as PE matmuls that
                # accumulate r1wT-stationary partial logits over the x chunks
                # as each DMA half lands (pool1 never materializes) ----
                ps_r = ppool.tile([4, 512], f32, tag="sps", name=f"ps_r1_{s}")
                for j in range(2):
                    for g in range(NCHUNK):
                        nc.tensor.matmul(
                            ps_r[:, 0:CHUNK],
                            r1wt_sb[:, j * NE : (j + 1) * NE],
                            x8[:, j, g * CHUNK : (g + 1) * CHUNK],
                            start=(j == 0 and g == 0),
                            stop=(j == 1 and g == NCHUNK - 1),
                        )
                p1log = spool.tile([4, 1], f32, tag="p1log")
                nc.vector.reduce_sum(p1log[:], ps_r[:, 0:CHUNK], AX)
                r1s = spool.tile([4, 1], f32, tag="r1s")
                nc.vector.tensor_tensor(r1s[:], p1log[:], r1b_col, op=ALU.add)
                nc.scalar.activation(r1s[:], r1s[:], AF.Sigmoid)
                d4 = spool.tile([4, 4], f32, tag="d4")
                nc.vector.tensor_scalar(d4[:], id4_sb, r1s[:], None, ALU.mult)
                ps_rb = ppool.tile([128, NE], f32, tag="sps", name=f"ps_rb1_{s}")
                nc.tensor.matmul(ps_rb[:], ones4_sb, d4[:], start=True, stop=True)

                # ---- mix k1T (DVE, bf16) ----
                k1t = spool.tile([128, 2, 128], bf16, tag="k1t")
                for j in range(2):
                    dst = k1t[:, j, :]
                    w_of = lambda e: w1t_sb[:, (j * NE + e) * 128 : (j * NE + e + 1) * 128]
                    nc.vector.tensor_scalar(dst, w_of(0), ps_rb[:, 0:1], None, ALU.mult)
                    for e in range(1, NE):
                        nc.vector.scalar_tensor_tensor(
                            dst, w_of(e), ps_rb[:, e : e + 1], dst, ALU.mult, ALU.add
                        )
                return k1t

            def stageC(s, x8, k1t):
                """conv1 (PE bf16) + ACT evac (fused bias+relu+pool2-accum)
                into the padded bf16 image; x1 out; fp8 converts."""
                pp = s % 2
                xpb = xpadb[pp]
                xpb_r = xpb[:].rearrange("p (r c) -> p r c", c=PADW)
                p2cols = spool.tile([128, NCHUNK], f32, tag="p2cols")
                for g in range(NCHUNK):
                    ps = cpool.tile([128, 512], f32, tag="cps", name=f"c1_{s}_{g}")
                    for j in range(2):
                        nc.tensor.matmul(
                            ps[:, 0:CHUNK],
                            k1t[:, j, :],
                            x8[:, j, g * CHUNK : (g + 1) * CHUNK],
                            start=(j == 0), stop=(j == 1),
                        )
                    nc.scalar.activation(
                        xpb_r[:, 1 + 8 * g : 9 + 8 * g, 1:57],
                        ps[:, 0:CHUNK].rearrange("p (r c) -> p r c", c=56),
                        AF.Relu, bias=bnb1_sb, accum_out=p2cols[:, g : g + 1],
                    )
                # x1 output DMA straight from the padded image (host strips);
                # issued from the ACT queue so it sits behind the evacs it
                # depends on and never blocks SP loads.
                nc.scalar.dma_start(out1_d[s], xpb[:, PADW : PADW + PSPAN])
                # ---- convert padded x1 to fp8 for the depthwise (GPSIMD) ----
                # two halves so dw(s) can start on the first rows earlier
                xp8 = xpad8[pp]
                nc.gpsimd.tensor_copy(
                    xp8[:, G8 + PADW : G8 + PADW + 4 * PCH],
                    xpb[:, PADW : PADW + 4 * PCH],
                )
                nc.gpsimd.tensor_copy(
                    xp8[:, G8 + PADW + 4 * PCH : G8 + PADW + PSPAN],
                    xpb[:, PADW + 4 * PCH : PADW + PSPAN],
                )
                return p2cols

            def stageR2(s, p2c):
                """routing2 -> k2 mix -> diag builds (no convert dependency)."""
                pp = s % 2
                # ---- routing 2 off the evac accum columns ----
                p2cols = p2c
                p2a = spool.tile([128, 1], f32, tag="p2a")
                nc.vector.reduce_sum(p2a[:], p2cols[:], AX)
                ps_r2 = ppool.tile([128, NE], f32, tag="sps", name=f"ps_r2_{s}")
                nc.tensor.matmul(ps_r2[0:1, :], p2a[:], r2wt_sb, start=True, stop=True)
                r2s = spool.tile([1, NE], f32, tag="r2s")
                nc.vector.tensor_tensor(r2s[:], ps_r2[0:1, :], r2b_sb, op=ALU.add)
                nc.scalar.activation(r2s[:], r2s[:], AF.Sigmoid)
                ps_rb2 = ppool.tile([128, NE], f32, tag="sps", name=f"ps_rb2_{s}")
                nc.tensor.matmul(ps_rb2[:], ones1_sb, r2s[:], start=True, stop=True)

                # ---- mix k2 and build fp8 diag pairs (DVE) ----
                k2 = spool.tile([128, 9], f32, tag="k2")
                nc.vector.tensor_scalar(k2[:], w2f_sb[:, 0:9], ps_rb2[:, 0:1], None, ALU.mult)
                for e in range(1, NE):
                    nc.vector.scalar_tensor_tensor(
                        k2[:], w2f_sb[:, e * 9 : (e + 1) * 9], ps_rb2[:, e : e + 1],
                        k2[:], ALU.mult, ALU.add,
                    )
                dg = diag[pp]
                for slot, t in enumerate(_SLOT_TAPS):
                    nc.vector.tensor_scalar(
                        dg[:, slot, :], id8_sb[:], k2[:, t : t + 1], None, ALU.mult
                    )

            def stageB(s):
                pp = s % 2
                xp8 = xpad8[pp]
                dg = diag[pp]
                xt = xp8[:]
                pitch = xt.ap[0][0]
                base = xt.offset + G8 + PADW
                x2 = mpool.tile([128, PSPAN], fp8e3, tag="x2")
                for g in range(NCHUNK):
                    ps = dpool.tile([128, 512], f32, tag="dps", name=f"dw_{s}_{g}")
                    for q, (slot, t0, d) in enumerate(_PAIRS):
                        rhs = RAP(
                            xt.tensor, base + g * PCH + _OFFS[t0],
                            [[pitch, 128], [d, 2], [1, PCH]],
                        )
                        nc.tensor.matmul(
                            ps[:, 0:PCH], dg[:, slot : slot + 2, :], rhs,
                            start=(q == 0), stop=(q == len(_PAIRS) - 1),
                            perf_mode=PM.DoubleRow,
                        )
                    if g < 5:
                        nc.scalar.activation(
                            x2[:, g * PCH : (g + 1) * PCH], ps[:, 0:PCH],
                            AF.Relu, bias=bnb2_sb,
                        )
                    else:
                        nc.vector.tensor_scalar(
                            x2[:, g * PCH : (g + 1) * PCH], ps[:, 0:PCH],
                            bnb2_sb, 0.0, ALU.add, ALU.max,
                        )
                # issue from ACT so it can't block SP loads; two pieces so
                # the first can fly while the tail chunks evacuate
                nc.scalar.dma_start(out2_d[s, :, 0 : 5 * PCH], x2[:, 0 : 5 * PCH])
                nc.scalar.dma_start(out2_d[s, :, 5 * PCH :], x2[:, 5 * PCH :])

            # Emission order doubles as each in-order engine queue's program
            # order (the tile scheduler mostly preserves it), so blocks are
            # laid out to overlap samples: sample s+1's pool1/conv1 land on
            # PE before sample s's routing2/dw, which wait on s's evacs.
            x8s, k1ts, p2s = {}, {}, {}
            x8s[0] = stageL(0)
            k1ts[0] = stageP1(0, x8s[0])
            for s in range(SPB):
                if s + 1 < SPB:
                    x8s[s + 1] = stageL(s + 1)
                p2s[s] = stageC(s, x8s[s], k1ts[s])
                if s + 1 < SPB:
                    k1ts[s + 1] = stageP1(s + 1, x8s[s + 1])
                stageR2(s, p2s[s])
                if s >= 1:
                    stageB(s - 1)
            stageB(SPB - 1)

    return nc


def _host_prep(x, r1_w, r1_b, w1, g1, b1, m1, v1, r2_w, r2_b, w2, g2, b2, m2, v2):
    import ml_dtypes

    inv1 = g1 / np.sqrt(v1 + BN_EPS)
    inv2 = g2 / np.sqrt(v2 + BN_EPS)
    bnb1 = (b1 - m1 * inv1).astype(np.float32)
    bnb2 = (b2 - m2 * inv2).astype(np.float32)
    # w1: [E, O, C, 1, 1] -> fold inv1 over O -> w1t[j, e, c_local, o]
    w1s = w1[:, :, :, 0, 0] * inv1[None, :, None]  # [E, O, C]
    w1t = np.ascontiguousarray(
        w1s.transpose(2, 0, 1).reshape(2, 128, NE, 128).transpose(0, 2, 1, 3)
    ).astype(ml_dtypes.bfloat16)  # [2, E, 128c, 128o]
    # w2: [E, C, 1, 3, 3] -> fold inv2 over C -> [c, (e 9)]
    w2f = (w2[:, :, 0, :, :] * inv2[None, :, None, None]).reshape(NE, EXP_C, 9)
    wpack = np.zeros((128, 315), dtype=np.float32)
    wpack[:, 0:36] = w2f.transpose(1, 0, 2).reshape(128, 36)
    wpack[:, 36:40] = np.ascontiguousarray(r2_w.T / HW).astype(np.float32)
    wpack[:, 44] = bnb1
    wpack[:, 45] = bnb2
    wpack[0:4, 46] = r1_b.astype(np.float32)
    wpack[0, 47:175] = 1.0
    wpack[0, 175:179] = r2_b.astype(np.float32)
    wpack[0:4, 183:311] = 1.0
    wpack[0:4, 311:315] = np.eye(4, dtype=np.float32)
    r1wt = np.ascontiguousarray(r1_w.T / HW)  # [256, 4]
    r1wt8 = np.concatenate(
        [r1wt[0:128], r1wt[128:256], r2_w.T / HW], axis=1
    )  # [128, 3*NE]
    common = {
        "w1t": w1t,
        "r1wt": r1wt8.astype(ml_dtypes.bfloat16),
        "wpack": wpack,
        "id8": np.eye(128, dtype=np.float32).astype(ml_dtypes.float8_e4m3),
    }
    return common


def _unpad(a):
    """[SPB, C, 3248] padded-row layout -> [SPB, C, 56, 56] f32."""
    a = np.asarray(a).astype(np.float32).reshape(a.shape[0], a.shape[1], 56, PADW)
    return a[:, :, :, 1:57]


def kernel(**inputs):
    import ml_dtypes

    x = np.asarray(inputs["x"], dtype=np.float32)
    common = _host_prep(**{k: np.asarray(v) for k, v in inputs.items()})

    if "nc" not in _prog_cache:
        _prog_cache["nc"] = _build_program()
    nc = _prog_cache["nc"]
    sim_mode = bool(os.environ.get("BASS_KERNEL_SIM"))
    if not sim_mode and not _prog_cache.get("fixed"):
        _legalize_sync(nc)
        _prog_cache["fixed"] = True

    xs = x.reshape(NCORES, SPB, CIN, HW).astype(ml_dtypes.bfloat16)
    in_maps = [dict(common, x=np.ascontiguousarray(xs[c])) for c in range(NCORES)]

    if sim_mode:
        from concourse.bass_interp import CoreSim

        sim = CoreSim(nc)
        for name, arr in in_maps[0].items():
            sim.tensor(name)[:] = arr
        sim.simulate()
        out = np.zeros((NCORES, SPB, COUT, H, W), dtype=np.float32)
        out[0, :, 0:INIT_C] = _unpad(sim.tensor("out1"))
        out[0, :, INIT_C:COUT] = _unpad(sim.tensor("out2"))
        return out.reshape(B, COUT, H, W)

    from concourse.bass_utils import run_bass_kernel_spmd

    res = run_bass_kernel_spmd(nc, in_maps, list(range(NCORES)))
    _prog_cache["last_results"] = res
    out = np.zeros((NCORES, SPB, COUT, H, W), dtype=np.float32)
    for c in range(NCORES):
        out[c, :, 0:INIT_C] = _unpad(res.results[c]["out1"])
        out[c, :, INIT_C:COUT] = _unpad(res.results[c]["out2"])
    return out.reshape(B, COUT, H, W)
